# revision 5
# baseline (speedup 1.0000x reference)
"""Trainium2 Bass kernel for grouped-correlation cost volume (GwcNet style).

cost[b,g,d,h,w] = mean_{c in group g}( ref[b,c,h,w] * tgt[b,c,h,w-d] ), 0 if w<d

Hardcoded problem size: B=4, C=320, H=64, W=128, D=48, G=40 (cg=8), f32.
Sharding: 8 cores = (4 batches) x (2 halves of H). Each core computes its
[G, D, 32, W] shard; host reassembles.

Per-core pipeline (per h-block of 8 rows, per disparity d):
  - inputs DMA'd per 128-partition channel chunk, cast f32->bf16 on ScalarE
  - tgt staged into two zero-padded tiles (even/odd column parity) so the
    d-shift is always a 4B-aligned slice -> DVE tensor_mul runs in 2x mode
  - group-sum via 6 accumulating PE matmuls against a constant block-diagonal
    (1/8) matrix; two h-quads are packed into 80 PSUM partitions (one bank)
  - PSUM -> SBUF copy on ScalarE, then DMA to the DRAM output volume
"""

import os
import sys

if "/opt/trn_rl_repo" not in sys.path:
    sys.path.insert(0, "/opt/trn_rl_repo")

import numpy as np

B, C, H, W = 4, 320, 64, 128
D, G, CG = 48, 40, 8
NCORES = 8
Hc = H // 2   # 32 rows of h per core
HB = 8        # h-block per inner tile (two quads of 4)
NHB = Hc // HB
PADW_E = 176  # even-parity padded tgt width (data at cols [48, 176))
PADW_O = 178  # odd-parity padded tgt width (data at cols [49, 177))

_CHUNKS = [(0, 128), (128, 128), (256, 64)]  # (c0, csz) partition chunks of C=320

_CACHE = {}
LAST_RESULT = None  # BassKernelResults of the most recent run (for profiling)


def _make_ones():
    """Block-diagonal group-mean weights: [128, 3, 2, 2*G] bf16.

    ones[p, k, j, 40*j + g] = 1/8 when global channel (c0_k + p) is in group g.
    Quad j of an h-block writes PSUM partitions [40j, 40j+40).
    """
    import ml_dtypes

    ones = np.zeros((128, 3, 2, 2 * G), dtype=ml_dtypes.bfloat16)
    for k, (c0, csz) in enumerate(_CHUNKS):
        for p in range(csz):
            g = (c0 + p) // CG
            for j in range(2):
                ones[p, k, j, G * j + g] = 0.125
    return ones


def _build_nc():
    import concourse.bass as bass
    import concourse.mybir as mybir
    from concourse import bacc, tile

    nc = bacc.Bacc("TRN2", target_bir_lowering=False, debug=False)
    ref_d = nc.dram_tensor("ref", [C, Hc, W], mybir.dt.float32, kind="ExternalInput")
    tgt_d = nc.dram_tensor("tgt", [C, Hc, W], mybir.dt.float32, kind="ExternalInput")
    ones_d = nc.dram_tensor(
        "ones", [128, 3, 2, 2 * G], mybir.dt.bfloat16, kind="ExternalInput"
    )
    out_d = nc.dram_tensor("out", [G, D, Hc, W], mybir.dt.float32, kind="ExternalOutput")

    bf16 = mybir.dt.bfloat16
    f32 = mybir.dt.float32

    with tile.TileContext(nc) as tc:
        with (
            tc.tile_pool(name="const", bufs=1) as constp,
            tc.tile_pool(name="inp", bufs=2) as inp,
            tc.tile_pool(name="prodp", bufs=3) as prodp,
            tc.tile_pool(name="outp", bufs=3) as outp,
            tc.tile_pool(name="psum", bufs=4, space="PSUM") as psump,
        ):
            ones_sb = constp.tile([128, 3, 2, 2 * G], bf16)
            nc.sync.dma_start(ones_sb[:], ones_d[:])

            # Persistent padded tgt tiles, manually double-buffered across
            # h-blocks. Pad columns are zeroed once; the data region is
            # rewritten each h-block.
            tge = [
                constp.tile(
                    [128, 3, HB, PADW_E], bf16, tag=f"tge{i}", name=f"tge{i}"
                )
                for i in range(2)
            ]
            tgo = [
                constp.tile(
                    [128, 3, HB, PADW_O], bf16, tag=f"tgo{i}", name=f"tgo{i}"
                )
                for i in range(2)
            ]
            for t in tge:
                nc.vector.memset(t[:, :, :, 0:48], 0.0)
            for t in tgo:
                nc.vector.memset(t[:, :, :, 0:49], 0.0)
                nc.vector.memset(t[:, :, :, 177:178], 0.0)

            for hb in range(NHB):
                h0 = hb * HB
                te = tge[hb % 2]
                to = tgo[hb % 2]
                ref_bf = inp.tile([128, 3, HB, W], bf16, tag="ref_bf")
                for k, (c0, csz) in enumerate(_CHUNKS):
                    rf = inp.tile([128, HB, W], f32, tag=f"rf{k}")
                    tf = inp.tile([128, HB, W], f32, tag=f"tf{k}")
                    nc.sync.dma_start(rf[0:csz], ref_d[c0 : c0 + csz, h0 : h0 + HB, :])
                    nc.sync.dma_start(tf[0:csz], tgt_d[c0 : c0 + csz, h0 : h0 + HB, :])
                    nc.scalar.copy(ref_bf[0:csz, k], rf[0:csz])
                    nc.scalar.copy(te[0:csz, k, :, 48 : 48 + W], tf[0:csz])
                    nc.scalar.copy(to[0:csz, k, :, 49 : 49 + W], tf[0:csz])

                for d in range(D):
                    par = d & 1
                    tp = to if par else te
                    off = (49 if par else 48) - d
                    prod = prodp.tile([128, 3, HB, W], bf16, tag="prod")
                    nc.vector.tensor_mul(
                        prod[:, 0:2],
                        ref_bf[:, 0:2],
                        tp[:, 0:2, :, off : off + W],
                    )
                    nc.vector.tensor_mul(
                        prod[0:64, 2],
                        ref_bf[0:64, 2],
                        tp[0:64, 2, :, off : off + W],
                    )
                    ps = psump.tile([2 * G, 4, W], f32, tag="ps")
                    for j in range(2):
                        for k, (c0, csz) in enumerate(_CHUNKS):
                            nc.tensor.matmul(
                                ps[:],
                                ones_sb[0:csz, k, j, :],
                                prod[0:csz, k, 4 * j : 4 * j + 4, :],
                                start=(j == 0 and k == 0),
                                stop=(j == 1 and k == 2),
                            )
                    ob = outp.tile([2 * G, 4, W], f32, tag="ob")
                    nc.scalar.copy(ob[:], ps[:])
                    for j in range(2):
                        nc.sync.dma_start(
                            out_d[:, d, h0 + 4 * j : h0 + 4 * j + 4, :],
                            ob[G * j : G * j + G],
                        )
    nc.compile()
    return nc


def _get_built():
    if "nc" not in _CACHE:
        _CACHE["nc"] = _build_nc()
        _CACHE["ones"] = _make_ones()
    return _CACHE["nc"], _CACHE["ones"]


def _kernel_numpy(ref, tgt, maxdisp, num_group):
    """Host fallback — guaranteed-correct grouped correlation volume."""
    cg = C // num_group
    r = ref.reshape(B, num_group, cg, H, W)
    out = np.zeros((B, num_group, maxdisp, H, W), np.float32)
    for d in range(maxdisp):
        t = np.zeros_like(tgt)
        if d:
            t[..., d:] = tgt[..., : W - d]
        else:
            t[...] = tgt
        tg = t.reshape(B, num_group, cg, H, W)
        out[:, :, d] = (r * tg).mean(axis=2)
    return out


def _kernel_device(ref, tgt):
    global LAST_RESULT
    from concourse import bass_utils

    nc, ones = _get_built()
    in_maps = []
    for i in range(NCORES):
        b, hh = divmod(i, 2)
        h0 = hh * Hc
        in_maps.append(
            {
                "ref": np.ascontiguousarray(ref[b, :, h0 : h0 + Hc, :]),
                "tgt": np.ascontiguousarray(tgt[b, :, h0 : h0 + Hc, :]),
                "ones": ones,
            }
        )

    trace = bool(int(os.environ.get("KTRACE", "0")))
    res = bass_utils.run_bass_kernel_spmd(
        nc, in_maps, list(range(NCORES)), trace=trace
    )
    LAST_RESULT = res

    out = np.empty((B, G, D, H, W), dtype=np.float32)
    for i in range(NCORES):
        b, hh = divmod(i, 2)
        out[b, :, :, hh * Hc : (hh + 1) * Hc, :] = res.results[i]["out"]
    return out


def kernel(refimg_fea, targetimg_fea, maxdisp=48, num_group=40):
    ref = np.asarray(refimg_fea, dtype=np.float32)
    tgt = np.asarray(targetimg_fea, dtype=np.float32)
    assert ref.shape == (B, C, H, W) and tgt.shape == (B, C, H, W)
    assert int(maxdisp) == D and int(num_group) == G

    try:
        return _kernel_device(ref, tgt)
    except Exception as e:  # device/compile failure: never return garbage
        sys.stderr.write(f"kernel: device path failed ({e!r}); numpy fallback\n")
        return _kernel_numpy(ref, tgt, int(maxdisp), int(num_group))
